# revision 5
# baseline (speedup 1.0000x reference)
"""Trainium2 Bass kernel for nn_Encoder (point-cloud encoder with segment-mean).

Strategy: data-parallel over clouds across 8 NeuronCores. Each core runs a
feature-major fused pipeline: point MLP (fp32r matmuls on PE, bias+LeakyReLU
evictions on ScalarE), per-segment sums (strided reduce on VectorE), then the
segment mean is taken BEFORE the (linear) final point layer, so the last point
layer and the latent MLP run on 512 clouds instead of 131072 points.

Reference-semantics note: the oracle's `idx` is produced with int32 overflow,
which makes its searchsorted assign every point segment id 4096 — all points
are dropped by segment_sum and the oracle latent input is exactly zero. The
kernel reproduces the oracle's semantics exactly via two host-computed
per-cloud vectors applied on-chip:
  factor[b] — multiplies the on-chip per-cloud h2a segment sums
              (0 when the oracle drops all points; 1/counts[b] when the
              segment layout is the uniform contiguous one)
  c3[b]     — scales the final point-layer bias contribution, added back via a
              rank-1 accumulating matmul (members_b / counts_b)
so the heavy device pipeline is unchanged while the output matches the oracle.
"""
import numpy as np
import concourse.bass as bass
import concourse.mybir as mybir
from concourse.tile import TileContext
from concourse.bass_utils import run_bass_kernel_spmd

F32 = mybir.dt.float32
F32R = mybir.dt.float32r

N_CORES = 8
N_TOTAL = 1_048_576
B = 4096
N_C = N_TOTAL // N_CORES      # points per core
B_C = B // N_CORES            # clouds per core
SEG = N_TOTAL // B            # 256 points per (uniform) cloud
CHUNK = 1024                  # points processed per loop iteration
N_ITER = N_C // CHUNK

# ---- packed weight layout: column offsets inside the [128, WCOLS] array ----
_BLOCKS = [
    ("w0t", 4, 64),        # pw0.T
    ("w1t", 64, 128),      # pw1.T
    ("w2_0", 128, 128),    # pw2.T[:, :128]
    ("w2_1", 128, 128),    # pw2.T[:, 128:]
    ("w3_00", 128, 128), ("w3_10", 128, 128), ("w3_01", 128, 128), ("w3_11", 128, 128),
    ("b3r_0", 1, 128), ("b3r_1", 1, 128),    # pb3 halves as K=1 rows
    ("lw0_00", 128, 128), ("lw0_10", 128, 128), ("lw0_01", 128, 128), ("lw0_11", 128, 128),
    ("lw1_00", 128, 128), ("lw1_10", 128, 128), ("lw1_01", 128, 128), ("lw1_11", 128, 128),
    ("mw_00", 128, 128), ("mw_10", 128, 128), ("mw_01", 128, 128), ("mw_11", 128, 128),
    ("vw_00", 128, 128), ("vw_10", 128, 128), ("vw_01", 128, 128), ("vw_11", 128, 128),
]
_OFFS = {}
_WIDTH = {}
_c = 0
for _n, _r, _w in _BLOCKS:
    _OFFS[_n] = _c
    _WIDTH[_n] = _w
    _c += _w
WCOLS = _c
NBIAS = 14  # b0(dual64), b1, b2 lo/hi, [unused pair], lb0 lo/hi, lb1 lo/hi, mb lo/hi, vb lo/hi


def _split_multi_waits(nc):
    """This walrus build supports only one sync-wait per lowered instruction;
    split extra waits into preceding single-wait EventSemaphore NOPs."""
    ctr = 0
    for f in nc.m.functions:
        for blk in f.blocks:
            out = []
            changed = False
            for inst in blk.instructions:
                si = inst.sync_info
                waits = list(si.on_wait) if si is not None else []
                if len(waits) > 1:
                    for w in waits[:-1]:
                        ctr += 1
                        ev = mybir.InstEventSemaphore(
                            name=f"antwaitsplit-{ctr}", ins=[], outs=[],
                            sync_info=mybir.SyncInfo(on_wait=[w], on_update=[]),
                        )
                        ev.engine = inst.engine
                        out.append(ev)
                    inst.sync_info = mybir.SyncInfo(
                        on_wait=[waits[-1]], on_update=list(si.on_update))
                    changed = True
                out.append(inst)
            if changed:
                blk.instructions = out
    return ctr


def build_nc():
    nc = bass.Bass()
    LR = mybir.ActivationFunctionType.Lrelu
    COPY = mybir.ActivationFunctionType.Copy
    AX = mybir.AxisListType.X

    xt_d = nc.dram_tensor("xt", [4, N_C], F32R, kind="ExternalInput")
    w_d = nc.dram_tensor("wpack", [128, WCOLS], F32R, kind="ExternalInput")
    b_d = nc.dram_tensor("bpack", [128, NBIAS], F32, kind="ExternalInput")
    f_d = nc.dram_tensor("fpack", [128, 2 * B_C], F32R, kind="ExternalInput")
    o_d = nc.dram_tensor("outp", [128, 4 * B_C], F32, kind="ExternalOutput")

    with TileContext(nc) as tc:
        with (
            tc.tile_pool(name="wp", bufs=1) as wp,
            tc.tile_pool(name="xp", bufs=4) as xp,
            tc.tile_pool(name="ap", bufs=2) as ap,
            tc.tile_pool(name="sp", bufs=1) as spp,
        ):
            wt = wp.tile([128, WCOLS], F32R)
            bt = wp.tile([128, NBIAS], F32)
            ft = wp.tile([128, 2 * B_C], F32R)
            nc.sync.dma_start(wt[:, :], w_d[:, :])
            nc.sync.dma_start(bt[:, :], b_d[:, :])
            nc.sync.dma_start(ft[:, :], f_d[:, :])

            def W(name, rows=128):
                off = _OFFS[name]
                return wt[0:rows, off:off + _WIDTH[name]]

            segsum_lo = spp.tile([128, B_C], F32)
            segsum_hi = spp.tile([128, B_C], F32)

            with tc.tile_pool(name="psA", bufs=1, space="PSUM") as psA:
                for i in range(N_ITER):
                    xt_i = xp.tile([4, CHUNK], F32R, name="xt_i")
                    nc.sync.dma_start(xt_i[:, :], xt_d[:, i * CHUNK:(i + 1) * CHUNK])

                    p0 = psA.tile([64, CHUNK], F32, name="p0", tag="p0")
                    for q in range(CHUNK // 512):
                        nc.tensor.matmul(p0[:, q * 512:(q + 1) * 512], W("w0t", 4),
                                         xt_i[:, q * 512:(q + 1) * 512],
                                         start=True, stop=True)
                    h0a = ap.tile([64, CHUNK], F32R, name="h0a", tag="h0a")
                    nc.scalar.activation(h0a[:, :], p0[:, :], LR,
                                         bias=bt[0:64, 0:1], alpha=0.01)

                    p1 = psA.tile([128, CHUNK], F32, name="p1", tag="p1")
                    for q in range(CHUNK // 512):
                        nc.tensor.matmul(p1[:, q * 512:(q + 1) * 512], W("w1t", 64),
                                         h0a[:, q * 512:(q + 1) * 512],
                                         start=True, stop=True)
                    h1a = ap.tile([128, CHUNK], F32R, name="h1a", tag="h1a")
                    nc.scalar.activation(h1a[:, :], p1[:, :], LR,
                                         bias=bt[:, 1:2], alpha=0.01)

                    p2a = psA.tile([128, CHUNK], F32, name="p2a", tag="p2a")
                    p2b = psA.tile([128, CHUNK], F32, name="p2b", tag="p2b")
                    for q in range(CHUNK // 512):
                        nc.tensor.matmul(p2a[:, q * 512:(q + 1) * 512], W("w2_0"),
                                         h1a[:, q * 512:(q + 1) * 512],
                                         start=True, stop=True)
                        nc.tensor.matmul(p2b[:, q * 512:(q + 1) * 512], W("w2_1"),
                                         h1a[:, q * 512:(q + 1) * 512],
                                         start=True, stop=True)
                    h2lo = ap.tile([128, CHUNK], F32, name="h2lo", tag="h2lo")
                    h2hi = ap.tile([128, CHUNK], F32, name="h2hi", tag="h2hi")
                    nc.scalar.activation(h2lo[:, :], p2a[:, :], LR,
                                         bias=bt[:, 2:3], alpha=0.01)
                    nc.scalar.activation(h2hi[:, :], p2b[:, :], LR,
                                         bias=bt[:, 3:4], alpha=0.01)

                    g = CHUNK // SEG
                    nc.vector.reduce_sum(
                        segsum_lo[:, i * g:(i + 1) * g],
                        h2lo[:, :].rearrange("p (g s) -> p g s", s=SEG), axis=AX)
                    nc.vector.reduce_sum(
                        segsum_hi[:, i * g:(i + 1) * g],
                        h2hi[:, :].rearrange("p (g s) -> p g s", s=SEG), axis=AX)

            # ---- tail: scaled mean -> L3 (+bias*c3) -> latent MLP -> outputs ----
            with tc.tile_pool(name="psB", bufs=4, space="PSUM") as psB:
                sc_lo = spp.tile([128, B_C], F32R)
                sc_hi = spp.tile([128, B_C], F32R)
                nc.vector.tensor_mul(sc_lo[:, :], segsum_lo[:, :], ft[:, 0:B_C].bitcast(F32))
                nc.vector.tensor_mul(sc_hi[:, :], segsum_hi[:, :], ft[:, 0:B_C].bitcast(F32))
                c3row = ft[0:1, B_C:2 * B_C]

                def layer(pref, rhs_lo, rhs_hi, bias_lo_col, bias_hi_col, func,
                          out_dtype, out_lo=None, out_hi=None, extra_k1=None):
                    plo = psB.tile([128, B_C], F32, name=f"{pref}_plo", tag="pt")
                    phi = psB.tile([128, B_C], F32, name=f"{pref}_phi", tag="pt")
                    for p, m in ((plo, 0), (phi, 1)):
                        last_noext = extra_k1 is None
                        nc.tensor.matmul(p[:, :], W(f"{pref}_0{m}"), rhs_lo,
                                         start=True, stop=False)
                        nc.tensor.matmul(p[:, :], W(f"{pref}_1{m}"), rhs_hi,
                                         start=False, stop=last_noext)
                        if extra_k1 is not None:
                            nc.tensor.matmul(p[:, :], W(extra_k1[m], 1), c3row,
                                             start=False, stop=True)
                    if out_lo is None:
                        out_lo = ap.tile([128, B_C], out_dtype, name=f"{pref}_olo",
                                         tag=f"{pref}_olo")
                        out_hi = ap.tile([128, B_C], out_dtype, name=f"{pref}_ohi",
                                         tag=f"{pref}_ohi")
                    if func is COPY:
                        nc.scalar.activation(out_lo, plo[:, :], func)
                        nc.scalar.activation(out_hi, phi[:, :], func)
                    else:
                        nc.scalar.activation(out_lo, plo[:, :], func,
                                             bias=bt[:, bias_lo_col:bias_lo_col + 1],
                                             alpha=0.01)
                        nc.scalar.activation(out_hi, phi[:, :], func,
                                             bias=bt[:, bias_hi_col:bias_hi_col + 1],
                                             alpha=0.01)
                    return out_lo, out_hi

                m3_lo, m3_hi = layer("w3", sc_lo[:, :], sc_hi[:, :], 0, 0, COPY, F32R,
                                     extra_k1=("b3r_0", "b3r_1"))
                l0_lo, l0_hi = layer("lw0", m3_lo.bitcast(F32R), m3_hi.bitcast(F32R),
                                     6, 7, mybir.ActivationFunctionType.Lrelu, F32R)
                l1_lo, l1_hi = layer("lw1", l0_lo.bitcast(F32R), l0_hi.bitcast(F32R),
                                     8, 9, mybir.ActivationFunctionType.Lrelu, F32R)

                outt = spp.tile([128, 4 * B_C], F32)
                layer("mw", l1_lo.bitcast(F32R), l1_hi.bitcast(F32R), 10, 11,
                      mybir.ActivationFunctionType.Identity, F32,
                      out_lo=outt[:, 0:B_C], out_hi=outt[:, B_C:2 * B_C])
                layer("vw", l1_lo.bitcast(F32R), l1_hi.bitcast(F32R), 12, 13,
                      mybir.ActivationFunctionType.Identity, F32,
                      out_lo=outt[:, 2 * B_C:3 * B_C], out_hi=outt[:, 3 * B_C:4 * B_C])
                nc.sync.dma_start(o_d[:, :], outt[:, :])

    _split_multi_waits(nc)
    return nc


_NC_CACHE = None


def _get_nc():
    global _NC_CACHE
    if _NC_CACHE is None:
        _NC_CACHE = build_nc()
    return _NC_CACHE


def _pack_weights(pw0, pw1, pw2, pw3, lw0, lw1, mw, vw, pb3):
    wpack = np.zeros((128, WCOLS), np.float32)

    def put(name, arr):
        r, c = arr.shape
        wpack[0:r, _OFFS[name]:_OFFS[name] + c] = arr

    put("w0t", pw0.T)
    put("w1t", pw1.T)
    w2t = pw2.T
    put("w2_0", w2t[:, 0:128])
    put("w2_1", w2t[:, 128:256])
    put("b3r_0", pb3[None, 0:128])
    put("b3r_1", pb3[None, 128:256])
    for pref, wm in (("w3", pw3.T), ("lw0", lw0.T), ("lw1", lw1.T),
                     ("mw", mw.T), ("vw", vw.T)):
        for k in (0, 1):
            for m in (0, 1):
                put(f"{pref}_{k}{m}", wm[k * 128:(k + 1) * 128, m * 128:(m + 1) * 128])
    return wpack


def _pack_biases(pb0, pb1, pb2, lb0, lb1, mb, vb):
    bp = np.zeros((128, NBIAS), np.float32)
    bp[0:64, 0] = pb0
    bp[64:128, 0] = pb0
    bp[:, 1] = pb1
    for col, vec in zip((2, 6, 8, 10, 12), (pb2, lb0, lb1, mb, vb)):
        bp[:, col] = vec[0:128]
        bp[:, col + 1] = vec[128:256]
    return bp


def _reference_numpy(points, idx, pw0, pb0, pw1, pb1, pw2, pb2, pw3, pb3,
                     lw0, lb0, lw1, lb1, mw, mb, vw, vb):
    """Exact-semantics fallback for segment layouts the device path doesn't
    model (never taken for the staged problem)."""
    def lrelu(x):
        return np.where(x > 0, x, np.float32(0.01) * x)
    h = lrelu(points @ pw0.T + pb0)
    h = lrelu(h @ pw1.T + pb1)
    h = lrelu(h @ pw2.T + pb2)
    h = h @ pw3.T + pb3
    n, b = h.shape[0], idx.shape[0]
    seg = np.searchsorted(idx, np.arange(n).astype(idx.dtype), side="right")
    valid = (seg >= 0) & (seg < b)
    sums = np.zeros((b, h.shape[1]), np.float32)
    np.add.at(sums, seg[valid], h[valid])
    starts = np.concatenate([idx[:1] * 0, idx[:-1]])
    counts = (idx - starts).astype(np.float32)
    with np.errstate(all="ignore"):
        latent = sums / counts[:, None]
    latent = lrelu(latent @ lw0.T + lb0)
    latent = lrelu(latent @ lw1.T + lb1)
    return latent @ mw.T + mb, latent @ vw.T + vb


def kernel(points, idx, pw0, pb0, pw1, pb1, pw2, pb2, pw3, pb3,
           lw0, lb0, lw1, lb1, mw, mb, vw, vb):
    points = np.asarray(points, np.float32)
    idx = np.asarray(idx)
    (pw0, pb0, pw1, pb1, pw2, pb2, pw3, pb3,
     lw0, lb0, lw1, lb1, mw, mb, vw, vb) = [
        np.asarray(a, np.float32) for a in
        (pw0, pb0, pw1, pb1, pw2, pb2, pw3, pb3, lw0, lb0, lw1, lb1, mw, mb, vw, vb)]

    n, b = points.shape[0], idx.shape[0]
    # replicate the oracle's segment assignment (including any idx overflow)
    seg = np.searchsorted(idx, np.arange(n).astype(idx.dtype), side="right")
    starts = np.concatenate([idx[:1] * 0, idx[:-1]])
    counts = (idx - starts).astype(np.float32)
    uniform_layout = (n == N_TOTAL and b == B and
                      np.array_equal(seg, np.arange(n) // SEG) and
                      np.all(counts == SEG))
    all_dropped = bool(np.all((seg < 0) | (seg >= b))) and n == N_TOTAL and b == B

    if uniform_layout:
        factor = (np.float32(1.0) / counts).astype(np.float32)      # 1/256
        c3 = np.ones(B, np.float32)
    elif all_dropped:
        factor = np.zeros(B, np.float32)
        c3 = np.zeros(B, np.float32)
    else:
        return _reference_numpy(points, idx, pw0, pb0, pw1, pb1, pw2, pb2, pw3,
                                pb3, lw0, lb0, lw1, lb1, mw, mb, vw, vb)

    xt = np.ascontiguousarray(points.T)              # [4, N_TOTAL]
    wpack = _pack_weights(pw0, pw1, pw2, pw3, lw0, lw1, mw, vw, pb3)
    bpack = _pack_biases(pb0, pb1, pb2, lb0, lb1, mb, vb)

    in_maps = []
    for c in range(N_CORES):
        fpack = np.zeros((128, 2 * B_C), np.float32)
        fpack[:, 0:B_C] = factor[c * B_C:(c + 1) * B_C][None, :]
        fpack[0, B_C:2 * B_C] = c3[c * B_C:(c + 1) * B_C]
        in_maps.append({
            "xt": np.ascontiguousarray(xt[:, c * N_C:(c + 1) * N_C]),
            "wpack": wpack,
            "bpack": bpack,
            "fpack": fpack,
        })

    nc = _get_nc()
    res = run_bass_kernel_spmd(nc, in_maps, core_ids=list(range(N_CORES)))

    mu = np.empty((B, 256), np.float32)
    lv = np.empty((B, 256), np.float32)
    for c in range(N_CORES):
        o = res.results[c]["outp"]
        sl = slice(c * B_C, (c + 1) * B_C)
        mu[sl, 0:128] = o[:, 0:B_C].T
        mu[sl, 128:256] = o[:, B_C:2 * B_C].T
        lv[sl, 0:128] = o[:, 2 * B_C:3 * B_C].T
        lv[sl, 128:256] = o[:, 3 * B_C:4 * B_C].T
    return mu, lv


# revision 8
# speedup vs baseline: 1.1819x; 1.1819x over previous
"""Trainium2 Bass kernel for nn_Encoder (point-cloud encoder with segment-mean).

Strategy: data-parallel over clouds across 8 NeuronCores. Each core runs a
feature-major fused pipeline: point MLP (fp32r matmuls on PE, bias+LeakyReLU
evictions on ScalarE), per-segment sums (strided reduce on VectorE), then the
segment mean is taken BEFORE the (linear) final point layer, so the last point
layer and the latent MLP run on 512 clouds instead of 131072 points.

Reference-semantics note: the oracle's `idx` is produced with int32 overflow,
which makes its searchsorted assign every point segment id 4096 — all points
are dropped by segment_sum and the oracle latent input is exactly zero. The
kernel reproduces the oracle's semantics exactly via two host-computed
per-cloud vectors applied on-chip:
  factor[b] — multiplies the on-chip per-cloud h2a segment sums
              (0 when the oracle drops all points; 1/counts[b] when the
              segment layout is the uniform contiguous one)
  c3[b]     — scales the final point-layer bias contribution, added back via a
              rank-1 accumulating matmul (members_b / counts_b)
so the heavy device pipeline is unchanged while the output matches the oracle.
"""
import numpy as np
import concourse.bass as bass
import concourse.mybir as mybir
from concourse.tile import TileContext
from concourse.bass_utils import run_bass_kernel_spmd

F32 = mybir.dt.float32
F32R = mybir.dt.float32r

N_CORES = 8
N_TOTAL = 1_048_576
B = 4096
N_C = N_TOTAL // N_CORES      # points per core
B_C = B // N_CORES            # clouds per core
SEG = N_TOTAL // B            # 256 points per (uniform) cloud
CHUNK = 1024                  # points processed per loop iteration
N_ITER = N_C // CHUNK
N_REPS = 1  # benchmark-only loop amplification; leave at 1

# ---- packed weight layout: column offsets inside the [128, WCOLS] array ----
_BLOCKS = [
    ("w0t", 4, 64),        # pw0.T
    ("w1t", 64, 128),      # pw1.T
    ("w2_0", 128, 128),    # pw2.T[:, :128]
    ("w2_1", 128, 128),    # pw2.T[:, 128:]
    ("w3_00", 128, 128), ("w3_10", 128, 128), ("w3_01", 128, 128), ("w3_11", 128, 128),
    ("b3r_0", 1, 128), ("b3r_1", 1, 128),    # pb3 halves as K=1 rows
    ("lw0_00", 128, 128), ("lw0_10", 128, 128), ("lw0_01", 128, 128), ("lw0_11", 128, 128),
    ("lw1_00", 128, 128), ("lw1_10", 128, 128), ("lw1_01", 128, 128), ("lw1_11", 128, 128),
    ("mw_00", 128, 128), ("mw_10", 128, 128), ("mw_01", 128, 128), ("mw_11", 128, 128),
    ("vw_00", 128, 128), ("vw_10", 128, 128), ("vw_01", 128, 128), ("vw_11", 128, 128),
]
_OFFS = {}
_WIDTH = {}
_c = 0
for _n, _r, _w in _BLOCKS:
    _OFFS[_n] = _c
    _WIDTH[_n] = _w
    _c += _w
WCOLS = _c
NBIAS = 14  # b0(dual64), b1, b2 lo/hi, [unused pair], lb0 lo/hi, lb1 lo/hi, mb lo/hi, vb lo/hi


def _split_multi_waits(nc):
    """This walrus build supports only one sync-wait per lowered instruction;
    split extra waits into preceding single-wait EventSemaphore NOPs."""
    ctr = 0
    for f in nc.m.functions:
        for blk in f.blocks:
            out = []
            changed = False
            for inst in blk.instructions:
                si = inst.sync_info
                waits = list(si.on_wait) if si is not None else []
                if len(waits) > 1:
                    for w in waits[:-1]:
                        ctr += 1
                        ev = mybir.InstEventSemaphore(
                            name=f"antwaitsplit-{ctr}", ins=[], outs=[],
                            sync_info=mybir.SyncInfo(on_wait=[w], on_update=[]),
                        )
                        ev.engine = inst.engine
                        out.append(ev)
                    inst.sync_info = mybir.SyncInfo(
                        on_wait=[waits[-1]], on_update=list(si.on_update))
                    changed = True
                out.append(inst)
            if changed:
                blk.instructions = out
    return ctr


def build_nc():
    nc = bass.Bass()
    LR = mybir.ActivationFunctionType.Lrelu
    COPY = mybir.ActivationFunctionType.Copy
    AX = mybir.AxisListType.X

    xt_d = nc.dram_tensor("xt", [4, N_C], F32R, kind="ExternalInput")
    w_d = nc.dram_tensor("wpack", [128, WCOLS], F32R, kind="ExternalInput")
    b_d = nc.dram_tensor("bpack", [128, NBIAS], F32, kind="ExternalInput")
    f_d = nc.dram_tensor("fpack", [128, 2 * B_C], F32R, kind="ExternalInput")
    o_d = nc.dram_tensor("outp", [128, 4 * B_C], F32, kind="ExternalOutput")

    with TileContext(nc) as tc:
        with (
            tc.tile_pool(name="wp", bufs=1) as wp,
            tc.tile_pool(name="xp", bufs=4) as xp,
            tc.tile_pool(name="ap", bufs=2) as ap,
            tc.tile_pool(name="sp", bufs=1) as spp,
        ):
            wt = wp.tile([128, WCOLS], F32R)
            bt = wp.tile([128, NBIAS], F32)
            ft = wp.tile([128, 2 * B_C], F32R)
            nc.sync.dma_start(wt[:, :], w_d[:, :])
            nc.sync.dma_start(bt[:, :], b_d[:, :])
            nc.sync.dma_start(ft[:, :], f_d[:, :])

            def W(name, rows=128):
                off = _OFFS[name]
                return wt[0:rows, off:off + _WIDTH[name]]

            segsum_lo = spp.tile([128, B_C], F32)
            segsum_hi = spp.tile([128, B_C], F32)

            with tc.tile_pool(name="psA", bufs=1, space="PSUM") as psA:
                for it in range(N_ITER * N_REPS):
                    i = it % N_ITER
                    xt_i = xp.tile([4, CHUNK], F32R, name="xt_i")
                    nc.sync.dma_start(xt_i[:, :], xt_d[:, i * CHUNK:(i + 1) * CHUNK])

                    p0 = psA.tile([64, CHUNK], F32, name="p0", tag="p0")
                    for q in range(CHUNK // 512):
                        nc.tensor.matmul(p0[:, q * 512:(q + 1) * 512], W("w0t", 4),
                                         xt_i[:, q * 512:(q + 1) * 512],
                                         start=True, stop=True)
                    h0a = ap.tile([64, CHUNK], F32R, name="h0a", tag="h0a")
                    nc.scalar.activation(h0a[:, :], p0[:, :], LR,
                                         bias=bt[0:64, 0:1], alpha=0.01)

                    p1 = psA.tile([128, CHUNK], F32, name="p1", tag="p1")
                    for q in range(CHUNK // 512):
                        nc.tensor.matmul(p1[:, q * 512:(q + 1) * 512], W("w1t", 64),
                                         h0a[:, q * 512:(q + 1) * 512],
                                         start=True, stop=True)
                    h1a = ap.tile([128, CHUNK], F32R, name="h1a", tag="h1a")
                    nc.scalar.activation(h1a[:, :], p1[:, :], LR,
                                         bias=bt[:, 1:2], alpha=0.01)

                    p2a = psA.tile([128, CHUNK], F32, name="p2a", tag="p2a")
                    p2b = psA.tile([128, CHUNK], F32, name="p2b", tag="p2b")
                    for q in range(CHUNK // 512):
                        nc.tensor.matmul(p2a[:, q * 512:(q + 1) * 512], W("w2_0"),
                                         h1a[:, q * 512:(q + 1) * 512],
                                         start=True, stop=True)
                        nc.tensor.matmul(p2b[:, q * 512:(q + 1) * 512], W("w2_1"),
                                         h1a[:, q * 512:(q + 1) * 512],
                                         start=True, stop=True)
                    h2lo = ap.tile([128, CHUNK], F32, name="h2lo", tag="h2lo")
                    h2hi = ap.tile([128, CHUNK], F32, name="h2hi", tag="h2hi")
                    nc.scalar.activation(h2lo[:, :], p2a[:, :], LR,
                                         bias=bt[:, 2:3], alpha=0.01)
                    nc.scalar.activation(h2hi[:, :], p2b[:, :], LR,
                                         bias=bt[:, 3:4], alpha=0.01)

                    g = CHUNK // SEG
                    nc.vector.reduce_sum(
                        segsum_lo[:, i * g:(i + 1) * g],
                        h2lo[:, :].rearrange("p (g s) -> p g s", s=SEG), axis=AX)
                    nc.vector.reduce_sum(
                        segsum_hi[:, i * g:(i + 1) * g],
                        h2hi[:, :].rearrange("p (g s) -> p g s", s=SEG), axis=AX)

            # ---- tail: scaled mean -> L3 (+bias*c3) -> latent MLP -> outputs ----
            with tc.tile_pool(name="psB", bufs=4, space="PSUM") as psB:
                sc_lo = spp.tile([128, B_C], F32R)
                sc_hi = spp.tile([128, B_C], F32R)
                nc.vector.tensor_mul(sc_lo[:, :], segsum_lo[:, :], ft[:, 0:B_C].bitcast(F32))
                nc.vector.tensor_mul(sc_hi[:, :], segsum_hi[:, :], ft[:, 0:B_C].bitcast(F32))
                c3row = ft[0:1, B_C:2 * B_C]

                def layer(pref, rhs_lo, rhs_hi, bias_lo_col, bias_hi_col, func,
                          out_dtype, out_lo=None, out_hi=None, extra_k1=None):
                    plo = psB.tile([128, B_C], F32, name=f"{pref}_plo", tag="pt")
                    phi = psB.tile([128, B_C], F32, name=f"{pref}_phi", tag="pt")
                    for p, m in ((plo, 0), (phi, 1)):
                        last_noext = extra_k1 is None
                        nc.tensor.matmul(p[:, :], W(f"{pref}_0{m}"), rhs_lo,
                                         start=True, stop=False)
                        nc.tensor.matmul(p[:, :], W(f"{pref}_1{m}"), rhs_hi,
                                         start=False, stop=last_noext)
                        if extra_k1 is not None:
                            nc.tensor.matmul(p[:, :], W(extra_k1[m], 1), c3row,
                                             start=False, stop=True)
                    if out_lo is None:
                        out_lo = ap.tile([128, B_C], out_dtype, name=f"{pref}_olo",
                                         tag=f"{pref}_olo")
                        out_hi = ap.tile([128, B_C], out_dtype, name=f"{pref}_ohi",
                                         tag=f"{pref}_ohi")
                    if func is COPY:
                        nc.scalar.activation(out_lo, plo[:, :], func)
                        nc.scalar.activation(out_hi, phi[:, :], func)
                    else:
                        nc.scalar.activation(out_lo, plo[:, :], func,
                                             bias=bt[:, bias_lo_col:bias_lo_col + 1],
                                             alpha=0.01)
                        nc.scalar.activation(out_hi, phi[:, :], func,
                                             bias=bt[:, bias_hi_col:bias_hi_col + 1],
                                             alpha=0.01)
                    return out_lo, out_hi

                m3_lo, m3_hi = layer("w3", sc_lo[:, :], sc_hi[:, :], 0, 0, COPY, F32R,
                                     extra_k1=("b3r_0", "b3r_1"))
                l0_lo, l0_hi = layer("lw0", m3_lo.bitcast(F32R), m3_hi.bitcast(F32R),
                                     6, 7, mybir.ActivationFunctionType.Lrelu, F32R)
                l1_lo, l1_hi = layer("lw1", l0_lo.bitcast(F32R), l0_hi.bitcast(F32R),
                                     8, 9, mybir.ActivationFunctionType.Lrelu, F32R)

                outt = spp.tile([128, 4 * B_C], F32)
                layer("mw", l1_lo.bitcast(F32R), l1_hi.bitcast(F32R), 10, 11,
                      mybir.ActivationFunctionType.Identity, F32,
                      out_lo=outt[:, 0:B_C], out_hi=outt[:, B_C:2 * B_C])
                layer("vw", l1_lo.bitcast(F32R), l1_hi.bitcast(F32R), 12, 13,
                      mybir.ActivationFunctionType.Identity, F32,
                      out_lo=outt[:, 2 * B_C:3 * B_C], out_hi=outt[:, 3 * B_C:4 * B_C])
                nc.sync.dma_start(o_d[:, :], outt[:, :])

    _split_multi_waits(nc)
    return nc


_NC_CACHE = None


def _get_nc():
    global _NC_CACHE
    if _NC_CACHE is None:
        _NC_CACHE = build_nc()
    return _NC_CACHE


def _pack_weights(pw0, pw1, pw2, pw3, lw0, lw1, mw, vw, pb3):
    wpack = np.zeros((128, WCOLS), np.float32)

    def put(name, arr):
        r, c = arr.shape
        wpack[0:r, _OFFS[name]:_OFFS[name] + c] = arr

    put("w0t", pw0.T)
    put("w1t", pw1.T)
    w2t = pw2.T
    put("w2_0", w2t[:, 0:128])
    put("w2_1", w2t[:, 128:256])
    put("b3r_0", pb3[None, 0:128])
    put("b3r_1", pb3[None, 128:256])
    for pref, wm in (("w3", pw3.T), ("lw0", lw0.T), ("lw1", lw1.T),
                     ("mw", mw.T), ("vw", vw.T)):
        for k in (0, 1):
            for m in (0, 1):
                put(f"{pref}_{k}{m}", wm[k * 128:(k + 1) * 128, m * 128:(m + 1) * 128])
    return wpack


def _pack_biases(pb0, pb1, pb2, lb0, lb1, mb, vb):
    bp = np.zeros((128, NBIAS), np.float32)
    bp[0:64, 0] = pb0
    bp[64:128, 0] = pb0
    bp[:, 1] = pb1
    for col, vec in zip((2, 6, 8, 10, 12), (pb2, lb0, lb1, mb, vb)):
        bp[:, col] = vec[0:128]
        bp[:, col + 1] = vec[128:256]
    return bp


def _reference_numpy(points, idx, pw0, pb0, pw1, pb1, pw2, pb2, pw3, pb3,
                     lw0, lb0, lw1, lb1, mw, mb, vw, vb):
    """Exact-semantics fallback for segment layouts the device path doesn't
    model (never taken for the staged problem)."""
    def lrelu(x):
        return np.where(x > 0, x, np.float32(0.01) * x)
    h = lrelu(points @ pw0.T + pb0)
    h = lrelu(h @ pw1.T + pb1)
    h = lrelu(h @ pw2.T + pb2)
    h = h @ pw3.T + pb3
    n, b = h.shape[0], idx.shape[0]
    seg = np.searchsorted(idx, np.arange(n).astype(idx.dtype), side="right")
    valid = (seg >= 0) & (seg < b)
    sums = np.zeros((b, h.shape[1]), np.float32)
    np.add.at(sums, seg[valid], h[valid])
    starts = np.concatenate([idx[:1] * 0, idx[:-1]])
    counts = (idx - starts).astype(np.float32)
    with np.errstate(all="ignore"):
        latent = sums / counts[:, None]
    latent = lrelu(latent @ lw0.T + lb0)
    latent = lrelu(latent @ lw1.T + lb1)
    return latent @ mw.T + mb, latent @ vw.T + vb


def kernel(points, idx, pw0, pb0, pw1, pb1, pw2, pb2, pw3, pb3,
           lw0, lb0, lw1, lb1, mw, mb, vw, vb):
    points = np.asarray(points, np.float32)
    idx = np.asarray(idx)
    (pw0, pb0, pw1, pb1, pw2, pb2, pw3, pb3,
     lw0, lb0, lw1, lb1, mw, mb, vw, vb) = [
        np.asarray(a, np.float32) for a in
        (pw0, pb0, pw1, pb1, pw2, pb2, pw3, pb3, lw0, lb0, lw1, lb1, mw, mb, vw, vb)]

    n, b = points.shape[0], idx.shape[0]
    # replicate the oracle's segment assignment (including any idx overflow)
    seg = np.searchsorted(idx, np.arange(n).astype(idx.dtype), side="right")
    starts = np.concatenate([idx[:1] * 0, idx[:-1]])
    counts = (idx - starts).astype(np.float32)
    uniform_layout = (n == N_TOTAL and b == B and
                      np.array_equal(seg, np.arange(n) // SEG) and
                      np.all(counts == SEG))
    all_dropped = bool(np.all((seg < 0) | (seg >= b))) and n == N_TOTAL and b == B

    if uniform_layout:
        factor = (np.float32(1.0) / counts).astype(np.float32)      # 1/256
        c3 = np.ones(B, np.float32)
    elif all_dropped:
        factor = np.zeros(B, np.float32)
        c3 = np.zeros(B, np.float32)
    else:
        return _reference_numpy(points, idx, pw0, pb0, pw1, pb1, pw2, pb2, pw3,
                                pb3, lw0, lb0, lw1, lb1, mw, mb, vw, vb)

    xt = np.ascontiguousarray(points.T)              # [4, N_TOTAL]
    wpack = _pack_weights(pw0, pw1, pw2, pw3, lw0, lw1, mw, vw, pb3)
    bpack = _pack_biases(pb0, pb1, pb2, lb0, lb1, mb, vb)

    in_maps = []
    for c in range(N_CORES):
        fpack = np.zeros((128, 2 * B_C), np.float32)
        fpack[:, 0:B_C] = factor[c * B_C:(c + 1) * B_C][None, :]
        fpack[0, B_C:2 * B_C] = c3[c * B_C:(c + 1) * B_C]
        in_maps.append({
            "xt": np.ascontiguousarray(xt[:, c * N_C:(c + 1) * N_C]),
            "wpack": wpack,
            "bpack": bpack,
            "fpack": fpack,
        })

    nc = _get_nc()
    res = run_bass_kernel_spmd(nc, in_maps, core_ids=list(range(N_CORES)))

    mu = np.empty((B, 256), np.float32)
    lv = np.empty((B, 256), np.float32)
    for c in range(N_CORES):
        o = res.results[c]["outp"]
        sl = slice(c * B_C, (c + 1) * B_C)
        mu[sl, 0:128] = o[:, 0:B_C].T
        mu[sl, 128:256] = o[:, B_C:2 * B_C].T
        lv[sl, 0:128] = o[:, 2 * B_C:3 * B_C].T
        lv[sl, 128:256] = o[:, 3 * B_C:4 * B_C].T
    return mu, lv


# revision 10
# speedup vs baseline: 1.1925x; 1.0090x over previous
"""Trainium2 Bass kernel for nn_Encoder (point-cloud encoder with segment-mean).

Strategy: data-parallel over clouds across 8 NeuronCores. Each core runs a
feature-major fused pipeline: point MLP (fp32r matmuls on PE, bias+LeakyReLU
evictions on ScalarE), per-segment sums (strided reduce on VectorE), then the
segment mean is taken BEFORE the (linear) final point layer, so the last point
layer and the latent MLP run on 512 clouds instead of 131072 points.

Reference-semantics note: the oracle's `idx` is produced with int32 overflow,
which makes its searchsorted assign every point segment id 4096 — all points
are dropped by segment_sum and the oracle latent input is exactly zero. The
kernel reproduces the oracle's semantics exactly via two host-computed
per-cloud vectors applied on-chip:
  factor[b] — multiplies the on-chip per-cloud h2a segment sums
              (0 when the oracle drops all points; 1/counts[b] when the
              segment layout is the uniform contiguous one)
  c3[b]     — scales the final point-layer bias contribution, added back via a
              rank-1 accumulating matmul (members_b / counts_b)
so the heavy device pipeline is unchanged while the output matches the oracle.
"""
import numpy as np
import concourse.bass as bass
import concourse.mybir as mybir
from concourse.tile import TileContext
from concourse.bass_utils import run_bass_kernel_spmd

F32 = mybir.dt.float32
F32R = mybir.dt.float32r

N_CORES = 8
N_TOTAL = 1_048_576
B = 4096
N_C = N_TOTAL // N_CORES      # points per core
B_C = B // N_CORES            # clouds per core
SEG = N_TOTAL // B            # 256 points per (uniform) cloud
CHUNK = 1024                  # points processed per loop iteration
N_ITER = N_C // CHUNK
N_REPS = 1  # benchmark-only loop amplification; leave at 1

# ---- packed weight layout: column offsets inside the [128, WCOLS] array ----
_BLOCKS = [
    ("w0t", 4, 64),        # pw0.T
    ("w1t", 64, 128),      # pw1.T
    ("w2_0", 128, 128),    # pw2.T[:, :128]
    ("w2_1", 128, 128),    # pw2.T[:, 128:]
    ("w3_00", 128, 128), ("w3_10", 128, 128), ("w3_01", 128, 128), ("w3_11", 128, 128),
    ("b3r_0", 1, 128), ("b3r_1", 1, 128),    # pb3 halves as K=1 rows
    ("lw0_00", 128, 128), ("lw0_10", 128, 128), ("lw0_01", 128, 128), ("lw0_11", 128, 128),
    ("lw1_00", 128, 128), ("lw1_10", 128, 128), ("lw1_01", 128, 128), ("lw1_11", 128, 128),
    ("mw_00", 128, 128), ("mw_10", 128, 128), ("mw_01", 128, 128), ("mw_11", 128, 128),
    ("vw_00", 128, 128), ("vw_10", 128, 128), ("vw_01", 128, 128), ("vw_11", 128, 128),
]
_OFFS = {}
_WIDTH = {}
_c = 0
for _n, _r, _w in _BLOCKS:
    _OFFS[_n] = _c
    _WIDTH[_n] = _w
    _c += _w
WCOLS = _c
NBIAS = 14  # b0(dual64), b1, b2 lo/hi, [unused pair], lb0 lo/hi, lb1 lo/hi, mb lo/hi, vb lo/hi


def _split_multi_waits(nc):
    """This walrus build supports only one sync-wait per lowered instruction;
    split extra waits into preceding single-wait EventSemaphore NOPs."""
    ctr = 0
    for f in nc.m.functions:
        for blk in f.blocks:
            out = []
            changed = False
            for inst in blk.instructions:
                si = inst.sync_info
                waits = list(si.on_wait) if si is not None else []
                if len(waits) > 1:
                    for w in waits[:-1]:
                        ctr += 1
                        ev = mybir.InstEventSemaphore(
                            name=f"antwaitsplit-{ctr}", ins=[], outs=[],
                            sync_info=mybir.SyncInfo(on_wait=[w], on_update=[]),
                        )
                        ev.engine = inst.engine
                        out.append(ev)
                    inst.sync_info = mybir.SyncInfo(
                        on_wait=[waits[-1]], on_update=list(si.on_update))
                    changed = True
                out.append(inst)
            if changed:
                blk.instructions = out
    return ctr


def build_nc():
    nc = bass.Bass()
    LR = mybir.ActivationFunctionType.Lrelu
    COPY = mybir.ActivationFunctionType.Copy
    AX = mybir.AxisListType.X

    xt_d = nc.dram_tensor("xt", [4, N_C], F32R, kind="ExternalInput")
    w_d = nc.dram_tensor("wpack", [128, WCOLS], F32R, kind="ExternalInput")
    b_d = nc.dram_tensor("bpack", [128, NBIAS], F32, kind="ExternalInput")
    f_d = nc.dram_tensor("fpack", [128, 2 * B_C], F32R, kind="ExternalInput")
    o_d = nc.dram_tensor("outp", [128, 4 * B_C], F32, kind="ExternalOutput")

    with TileContext(nc) as tc:
        with (
            tc.tile_pool(name="wp", bufs=1) as wp,
            tc.tile_pool(name="xp", bufs=4) as xp,
            tc.tile_pool(name="ap", bufs=2) as ap,
            tc.tile_pool(name="sp", bufs=1) as spp,
        ):
            wt = wp.tile([128, WCOLS], F32R)
            bt = wp.tile([128, NBIAS], F32)
            ft = wp.tile([128, 2 * B_C], F32R)
            nc.sync.dma_start(wt[:, :], w_d[:, :])
            nc.sync.dma_start(bt[:, :], b_d[:, :])
            nc.sync.dma_start(ft[:, :], f_d[:, :])

            def W(name, rows=128):
                off = _OFFS[name]
                return wt[0:rows, off:off + _WIDTH[name]]

            segsum_lo = spp.tile([128, B_C], F32)
            segsum_hi = spp.tile([128, B_C], F32)

            with tc.tile_pool(name="psA", bufs=1, space="PSUM") as psA:
                for it in range(N_ITER * N_REPS):
                    i = it % N_ITER
                    xt_i = xp.tile([4, CHUNK], F32R, name="xt_i")
                    nc.sync.dma_start(xt_i[:, :], xt_d[:, i * CHUNK:(i + 1) * CHUNK])

                    p0 = psA.tile([64, CHUNK], F32, name="p0", tag="p0")
                    for q in range(CHUNK // 512):
                        nc.tensor.matmul(p0[:, q * 512:(q + 1) * 512], W("w0t", 4),
                                         xt_i[:, q * 512:(q + 1) * 512],
                                         start=True, stop=True)
                    h0a = ap.tile([64, CHUNK], F32R, name="h0a", tag="h0a")
                    nc.scalar.activation(h0a[:, :], p0[:, :], LR,
                                         bias=bt[0:64, 0:1], alpha=0.01)

                    p1 = psA.tile([128, CHUNK], F32, name="p1", tag="p1")
                    for q in range(CHUNK // 512):
                        nc.tensor.matmul(p1[:, q * 512:(q + 1) * 512], W("w1t", 64),
                                         h0a[:, q * 512:(q + 1) * 512],
                                         start=True, stop=True)
                    h1a = ap.tile([128, CHUNK], F32R, name="h1a", tag="h1a")
                    nc.scalar.activation(h1a[:, :], p1[:, :], LR,
                                         bias=bt[:, 1:2], alpha=0.01)

                    p2a = psA.tile([128, CHUNK], F32, name="p2a", tag="p2a")
                    p2b = psA.tile([128, CHUNK], F32, name="p2b", tag="p2b")
                    for q in range(CHUNK // 512):
                        nc.tensor.matmul(p2a[:, q * 512:(q + 1) * 512], W("w2_0"),
                                         h1a[:, q * 512:(q + 1) * 512],
                                         start=True, stop=True)
                        nc.tensor.matmul(p2b[:, q * 512:(q + 1) * 512], W("w2_1"),
                                         h1a[:, q * 512:(q + 1) * 512],
                                         start=True, stop=True)
                    h2lo = ap.tile([128, CHUNK], F32, name="h2lo", tag="h2lo")
                    h2hi = ap.tile([128, CHUNK], F32, name="h2hi", tag="h2hi")
                    nc.scalar.activation(h2lo[:, :], p2a[:, :], LR,
                                         bias=bt[:, 2:3], alpha=0.01)
                    nc.scalar.activation(h2hi[:, :], p2b[:, :], LR,
                                         bias=bt[:, 3:4], alpha=0.01)

                    g = CHUNK // SEG
                    nc.vector.reduce_sum(
                        segsum_lo[:, i * g:(i + 1) * g],
                        h2lo[:, :].rearrange("p (g s) -> p g s", s=SEG), axis=AX)
                    nc.vector.reduce_sum(
                        segsum_hi[:, i * g:(i + 1) * g],
                        h2hi[:, :].rearrange("p (g s) -> p g s", s=SEG), axis=AX)

            # ---- tail: scaled mean -> L3 (+bias*c3) -> latent MLP -> outputs ----
            with tc.tile_pool(name="psB", bufs=4, space="PSUM") as psB:
                sc_lo = spp.tile([128, B_C], F32R)
                sc_hi = spp.tile([128, B_C], F32R)
                nc.vector.tensor_mul(sc_lo[:, :], segsum_lo[:, :], ft[:, 0:B_C].bitcast(F32))
                nc.vector.tensor_mul(sc_hi[:, :], segsum_hi[:, :], ft[:, 0:B_C].bitcast(F32))
                c3row = ft[0:1, B_C:2 * B_C]

                def layer(pref, rhs_lo, rhs_hi, bias_lo_col, bias_hi_col, func,
                          out_dtype, out_lo=None, out_hi=None, extra_k1=None):
                    plo = psB.tile([128, B_C], F32, name=f"{pref}_plo", tag="pt")
                    phi = psB.tile([128, B_C], F32, name=f"{pref}_phi", tag="pt")
                    for p, m in ((plo, 0), (phi, 1)):
                        last_noext = extra_k1 is None
                        nc.tensor.matmul(p[:, :], W(f"{pref}_0{m}"), rhs_lo,
                                         start=True, stop=False)
                        nc.tensor.matmul(p[:, :], W(f"{pref}_1{m}"), rhs_hi,
                                         start=False, stop=last_noext)
                        if extra_k1 is not None:
                            nc.tensor.matmul(p[:, :], W(extra_k1[m], 1), c3row,
                                             start=False, stop=True)
                    if out_lo is None:
                        out_lo = ap.tile([128, B_C], out_dtype, name=f"{pref}_olo",
                                         tag=f"{pref}_olo")
                        out_hi = ap.tile([128, B_C], out_dtype, name=f"{pref}_ohi",
                                         tag=f"{pref}_ohi")
                    if func is COPY:
                        nc.scalar.activation(out_lo, plo[:, :], func)
                        nc.scalar.activation(out_hi, phi[:, :], func)
                    else:
                        nc.scalar.activation(out_lo, plo[:, :], func,
                                             bias=bt[:, bias_lo_col:bias_lo_col + 1],
                                             alpha=0.01)
                        nc.scalar.activation(out_hi, phi[:, :], func,
                                             bias=bt[:, bias_hi_col:bias_hi_col + 1],
                                             alpha=0.01)
                    return out_lo, out_hi

                m3_lo, m3_hi = layer("w3", sc_lo[:, :], sc_hi[:, :], 0, 0, COPY, F32R,
                                     extra_k1=("b3r_0", "b3r_1"))
                l0_lo, l0_hi = layer("lw0", m3_lo.bitcast(F32R), m3_hi.bitcast(F32R),
                                     6, 7, mybir.ActivationFunctionType.Lrelu, F32R)
                l1_lo, l1_hi = layer("lw1", l0_lo.bitcast(F32R), l0_hi.bitcast(F32R),
                                     8, 9, mybir.ActivationFunctionType.Lrelu, F32R)

                outt = spp.tile([128, 4 * B_C], F32)
                layer("mw", l1_lo.bitcast(F32R), l1_hi.bitcast(F32R), 10, 11,
                      mybir.ActivationFunctionType.Identity, F32,
                      out_lo=outt[:, 0:B_C], out_hi=outt[:, B_C:2 * B_C])
                layer("vw", l1_lo.bitcast(F32R), l1_hi.bitcast(F32R), 12, 13,
                      mybir.ActivationFunctionType.Identity, F32,
                      out_lo=outt[:, 2 * B_C:3 * B_C], out_hi=outt[:, 3 * B_C:4 * B_C])
                nc.sync.dma_start(o_d[:, :], outt[:, :])

    _split_multi_waits(nc)
    return nc


_NC_CACHE = None


def _get_nc():
    global _NC_CACHE
    if _NC_CACHE is None:
        _NC_CACHE = build_nc()
    return _NC_CACHE


def _pack_weights(pw0, pw1, pw2, pw3, lw0, lw1, mw, vw, pb3):
    wpack = np.zeros((128, WCOLS), np.float32)

    def put(name, arr):
        r, c = arr.shape
        wpack[0:r, _OFFS[name]:_OFFS[name] + c] = arr

    put("w0t", pw0.T)
    put("w1t", pw1.T)
    w2t = pw2.T
    put("w2_0", w2t[:, 0:128])
    put("w2_1", w2t[:, 128:256])
    put("b3r_0", pb3[None, 0:128])
    put("b3r_1", pb3[None, 128:256])
    for pref, wm in (("w3", pw3.T), ("lw0", lw0.T), ("lw1", lw1.T),
                     ("mw", mw.T), ("vw", vw.T)):
        for k in (0, 1):
            for m in (0, 1):
                put(f"{pref}_{k}{m}", wm[k * 128:(k + 1) * 128, m * 128:(m + 1) * 128])
    return wpack


def _pack_biases(pb0, pb1, pb2, lb0, lb1, mb, vb, pw1):
    bp = np.zeros((128, NBIAS), np.float32)
    bp[0:64, 0] = pb0
    bp[64:128, 0] = pb0
    bp[:, 1] = pb1
    for col, vec in zip((2, 6, 8, 10, 12), (pb2, lb0, lb1, mb, vb)):
        bp[:, col] = vec[0:128]
        bp[:, col + 1] = vec[128:256]
    return bp


def _reference_numpy(points, idx, pw0, pb0, pw1, pb1, pw2, pb2, pw3, pb3,
                     lw0, lb0, lw1, lb1, mw, mb, vw, vb):
    """Exact-semantics fallback for segment layouts the device path doesn't
    model (never taken for the staged problem)."""
    def lrelu(x):
        return np.where(x > 0, x, np.float32(0.01) * x)
    h = lrelu(points @ pw0.T + pb0)
    h = lrelu(h @ pw1.T + pb1)
    h = lrelu(h @ pw2.T + pb2)
    h = h @ pw3.T + pb3
    n, b = h.shape[0], idx.shape[0]
    seg = np.searchsorted(idx, np.arange(n).astype(idx.dtype), side="right")
    valid = (seg >= 0) & (seg < b)
    sums = np.zeros((b, h.shape[1]), np.float32)
    np.add.at(sums, seg[valid], h[valid])
    starts = np.concatenate([idx[:1] * 0, idx[:-1]])
    counts = (idx - starts).astype(np.float32)
    with np.errstate(all="ignore"):
        latent = sums / counts[:, None]
    latent = lrelu(latent @ lw0.T + lb0)
    latent = lrelu(latent @ lw1.T + lb1)
    return latent @ mw.T + mb, latent @ vw.T + vb


def kernel(points, idx, pw0, pb0, pw1, pb1, pw2, pb2, pw3, pb3,
           lw0, lb0, lw1, lb1, mw, mb, vw, vb):
    points = np.asarray(points, np.float32)
    idx = np.asarray(idx)
    (pw0, pb0, pw1, pb1, pw2, pb2, pw3, pb3,
     lw0, lb0, lw1, lb1, mw, mb, vw, vb) = [
        np.asarray(a, np.float32) for a in
        (pw0, pb0, pw1, pb1, pw2, pb2, pw3, pb3, lw0, lb0, lw1, lb1, mw, mb, vw, vb)]

    n, b = points.shape[0], idx.shape[0]
    # replicate the oracle's segment assignment (including any idx overflow)
    seg = np.searchsorted(idx, np.arange(n).astype(idx.dtype), side="right")
    starts = np.concatenate([idx[:1] * 0, idx[:-1]])
    counts = (idx - starts).astype(np.float32)
    uniform_layout = (n == N_TOTAL and b == B and
                      np.array_equal(seg, np.arange(n) // SEG) and
                      np.all(counts == SEG))
    all_dropped = bool(np.all((seg < 0) | (seg >= b))) and n == N_TOTAL and b == B

    if uniform_layout:
        factor = (np.float32(1.0) / counts).astype(np.float32)      # 1/256
        c3 = np.ones(B, np.float32)
    elif all_dropped:
        factor = np.zeros(B, np.float32)
        c3 = np.zeros(B, np.float32)
    else:
        return _reference_numpy(points, idx, pw0, pb0, pw1, pb1, pw2, pb2, pw3,
                                pb3, lw0, lb0, lw1, lb1, mw, mb, vw, vb)

    xt = np.ascontiguousarray(points.T)              # [4, N_TOTAL]
    wpack = _pack_weights(pw0, pw1, pw2, pw3, lw0, lw1, mw, vw, pb3)
    bpack = _pack_biases(pb0, pb1, pb2, lb0, lb1, mb, vb, pw1)

    in_maps = []
    for c in range(N_CORES):
        fpack = np.zeros((128, 2 * B_C), np.float32)
        fpack[:, 0:B_C] = factor[c * B_C:(c + 1) * B_C][None, :]
        fpack[0, B_C:2 * B_C] = c3[c * B_C:(c + 1) * B_C]
        in_maps.append({
            "xt": np.ascontiguousarray(xt[:, c * N_C:(c + 1) * N_C]),
            "wpack": wpack,
            "bpack": bpack,
            "fpack": fpack,
        })

    nc = _get_nc()
    res = run_bass_kernel_spmd(nc, in_maps, core_ids=list(range(N_CORES)))

    mu = np.empty((B, 256), np.float32)
    lv = np.empty((B, 256), np.float32)
    for c in range(N_CORES):
        o = res.results[c]["outp"]
        sl = slice(c * B_C, (c + 1) * B_C)
        mu[sl, 0:128] = o[:, 0:B_C].T
        mu[sl, 128:256] = o[:, B_C:2 * B_C].T
        lv[sl, 0:128] = o[:, 2 * B_C:3 * B_C].T
        lv[sl, 128:256] = o[:, 3 * B_C:4 * B_C].T
    return mu, lv
